# revision 27
# baseline (speedup 1.0000x reference)
"""Trainium2 Bass kernel for nn_ActionVectorQuantizer.

Vector quantizer: for each row of z [262144, 256], find the nearest of 4
codebook rows e_k (squared-L2 argmin), output (z_q = e[idx], idx).

Sharding: data-parallel over the batch across 8 NeuronCores; the [4, 256]
codebook is replicated.  Each core handles 32768 rows with no communication.

Math per row r (all on-device, fp32):
    argmin_k ||z_r - e_k||^2  ==  argmax_k h_k   where
    h_0 = 0,  h_k = 2<z_r, e_k - e_0> - (||e_k||^2 - ||e_0||^2)   k = 1..3
The argmax is computed with the DVE max/max_index ops (first-index
tie-break, matching jnp.argmin).  z_q is produced on-device via a one-hot
[4,128] x [4,256] matmul on the tensor engine (split-bf16 codebook so the
gather is exact to ~1e-5 relative; PSUM -> SBUF copy on the scalar engine).
"""

import numpy as np

N_CORES = 8
BATCH = 262144
SHARD = BATCH // N_CORES          # 32768 rows per core
D = 256                           # code dim
K = 4                             # number of codes
P = 128                           # rows per tile (SBUF partitions)
TILES = SHARD // P                # 256 tiles per core
TB = 8                            # tiles per DMA batch (1 MiB per transfer)
NB = TILES // TB                  # 32 batches
NEG_BIG = -3.0e38

# Fraction of tile-batches routed through the PE (transpose+matmul) path
# instead of the DVE tensor_tensor_reduce path.  0 = all-DVE.
PE_PATH_NUM = 0
PE_PATH_DEN = 3


def build_nc(nb=NB):
    import concourse.bass as bass
    import concourse.bacc as bacc
    import concourse.mybir as mybir
    from concourse.tile import TileContext
    from contextlib import ExitStack

    f32 = mybir.dt.float32
    bf16 = mybir.dt.bfloat16
    u32 = mybir.dt.uint32

    shard = nb * TB * P
    nc = bacc.Bacc()
    z = nc.declare_dram_parameter("z", [shard, D], f32, isOutput=False)
    emb = nc.declare_dram_parameter("embedding", [K, D], f32, isOutput=False)
    zq = nc.declare_dram_parameter("z_q", [shard, D], f32, isOutput=True)
    idx = nc.declare_dram_parameter("idx", [shard], u32, isOutput=True)

    # DRAM views: batch b, tile t, partition p, dim d
    z_v = z.rearrange("(n t p) d -> n p t d", t=TB, p=P)        # [NB, 128, TB, D]
    zq_v = zq.rearrange("(n t p) d -> n p t d", t=TB, p=P)
    idx_v = idx.rearrange("(n c) -> n c", c=TB * P)             # [NB, 1024]

    with TileContext(nc) as tc, ExitStack() as ctx:
        const = ctx.enter_context(tc.tile_pool(name="const", bufs=1))
        io = ctx.enter_context(tc.tile_pool(name="io", bufs=3))
        work = ctx.enter_context(tc.tile_pool(name="work", bufs=2))
        idxp = ctx.enter_context(tc.tile_pool(name="idxp", bufs=4))
        psum = ctx.enter_context(tc.tile_pool(name="psum", bufs=4, space="PSUM"))

        # ---- constants -------------------------------------------------
        e_sb = const.tile([K, D], f32)
        nc.sync.dma_start(out=e_sb, in_=emb[:, :])

        # e rows broadcast to all 128 partitions
        e_bc = []
        for k in range(K):
            row = const.tile([1, D], f32, name=f"e_row{k}")
            nc.sync.dma_start(out=row, in_=emb[k : k + 1, :])
            t = const.tile([P, D], f32, name=f"e_bc{k}")
            nc.gpsimd.partition_broadcast(t, row)
            e_bc.append(t)

        # f_k = e_k - e_0 broadcast (k=1..3)
        f_bc = []
        for k in range(1, K):
            t = const.tile([P, D], f32, name=f"f_bc{k}")
            nc.vector.tensor_sub(t, e_bc[k], e_bc[0])
            f_bc.append(t)

        # ee[:, k] = ||e_k||^2 per partition; negg[:, k-1] = -(ee_k - ee_0)
        ee = const.tile([P, K], f32)
        scr_act = const.tile([P, D], f32)
        for k in range(K):
            nc.scalar.activation(
                out=scr_act,
                in_=e_bc[k],
                func=mybir.ActivationFunctionType.Square,
                accum_out=ee[:, k : k + 1],
            )
        # g_rep: (ee_k - ee_0) for k=1..3, repeated TB times along free
        gg = const.tile([P, K - 1], f32)
        nc.vector.tensor_scalar(
            gg, ee[:, 1:K], ee[:, 0:1], None, op0=mybir.AluOpType.subtract
        )
        g_rep = const.tile([P, (K - 1) * TB], f32)
        for t in range(TB):
            nc.vector.tensor_copy(g_rep[:, 3 * t : 3 * t + 3], gg)
        g_rep_v = g_rep.rearrange("p (t e) -> p t e", e=3)

        # split-bf16 codebook for the exact one-hot gather
        e_hi = const.tile([K, D], bf16)
        nc.vector.tensor_copy(e_hi, e_sb)
        e_hi_f = const.tile([K, D], f32)
        nc.vector.tensor_copy(e_hi_f, e_hi)
        e_lo_f = const.tile([K, D], f32)
        nc.vector.tensor_sub(e_lo_f, e_sb, e_hi_f)
        e_lo = const.tile([K, D], bf16)
        nc.vector.tensor_copy(e_lo, e_lo_f)

        # per-partition code id 0..3 (for the one-hot compare).  Engines
        # can't write at unaligned base partitions, so build [0,1,2,3] as a
        # row on partition 0 and PE-transpose it into a column.
        row4 = const.tile([1, K], f32)
        for k in range(K):
            nc.vector.memset(row4[:, k : k + 1], float(k))
        one1 = const.tile([1, 1], f32)
        nc.vector.memset(one1, 1.0)
        iota_ps = psum.tile([K, 1], f32, tag="iota_ps", bufs=1)
        nc.tensor.transpose(iota_ps, row4, one1)
        iota4 = const.tile([K, 1], f32)
        nc.vector.tensor_copy(iota4, iota_ps)

        # ---- main loop -------------------------------------------------
        for b in range(nb):
            z_b = io.tile([P, TB * D], f32, tag="z_b")
            nc.sync.dma_start(out=z_b, in_=z_v[b])

            zq_b = io.tile([P, TB * D], f32, tag="zq_b")

            h_b = work.tile([P, 8 * TB], f32, tag="h_b")
            nc.vector.memset(h_b, NEG_BIG)
            nc.vector.memset(
                h_b.rearrange("p (t e) -> p t e", e=8)[:, :, 0:1], 0.0
            )

            # vt holds the 8 max_index outputs packed for 32x32 transposes:
            # group g (4 tiles) occupies cols 32g..32g+32, tile slot j at 8j
            vt = idxp.tile([P, 8 * TB], u32, tag="vt")
            vt_t = idxp.tile([P, 8 * TB], u32, tag="vt_t")
            scr = work.tile([P, D], f32, tag="ttr_scr")
            mx = work.tile([P, 8], f32, tag="mx")

            # h_k = 2<z, e_k - e_0> - (ee_k - ee_0),  k = 1..3
            for t in range(TB):
                zt = z_b[:, t * D : (t + 1) * D]
                hs = h_b[:, 8 * t : 8 * t + 8]
                for k in range(1, K):
                    nc.vector.scalar_tensor_tensor(
                        out=scr,
                        in0=zt,
                        scalar=2.0,
                        in1=f_bc[k - 1],
                        op0=mybir.AluOpType.mult,
                        op1=mybir.AluOpType.mult,
                        accum_out=hs[:, k : k + 1],
                    )
            h_v = h_b.rearrange("p (t e) -> p t e", e=8)
            nc.vector.tensor_tensor(
                out=h_v[:, :, 1:4],
                in0=h_v[:, :, 1:4],
                in1=g_rep_v,
                op=mybir.AluOpType.subtract,
            )
            for t in range(TB):
                hs = h_b[:, 8 * t : 8 * t + 8]
                nc.vector.max(out=mx, in_=hs)
                g, j = t // 4, t % 4
                # write the 8 maxes at stride 4 so tile j's argmax (output
                # 0) lands in column 32g + j
                nc.vector.max_index(
                    out=vt[:, 32 * g + j : 32 * (g + 1) : 4],
                    in_max=mx,
                    in_values=hs,
                )

            # 32x32 block transpose: vt_t[32B + j, 32g + i] = argmax idx of
            # tile 4g+j, row 32B+i  (j < 4)
            nc.vector.transpose(vt_t, vt)

            # Assemble [1, 1024] in row order via plain-sliced DMAs, then
            # replicate across the 4 code partitions.
            idxt_row = idxp.tile([1, P * TB], u32, tag="idxt_row")
            idxt = idxp.tile([K, P * TB], u32, tag="idxt")
            dstv = idxt_row.rearrange(
                "k (g j B i) -> k g j B i", g=2, j=4, B=4, i=32
            )
            for g in range(2):
                for B in range(4):
                    nc.scalar.dma_start(
                        out=dstv[:, g : g + 1, :, B : B + 1, :],
                        in_=vt_t[32 * B : 32 * B + 4, 32 * g : 32 * g + 32],
                    )
            nc.gpsimd.partition_broadcast(idxt, idxt_row)

            # one-hot (bf16): ot[k, c] = (idxt[k, c] == k)
            ot = work.tile([K, P * TB], bf16, tag="ot")
            nc.vector.tensor_scalar(
                ot, idxt, iota4, None, op0=mybir.AluOpType.is_equal
            )

            # gather: zq_tile = onehot.T @ (e_hi + e_lo)
            for t in range(TB):
                zq_ps = psum.tile([P, D], f32, tag="zq_ps")
                lhsT = ot[:, P * t : P * (t + 1)]
                nc.tensor.matmul(zq_ps, lhsT, e_hi, start=True, stop=False)
                nc.tensor.matmul(zq_ps, lhsT, e_lo, start=False, stop=True)
                nc.scalar.copy(out=zq_b[:, t * D : (t + 1) * D], in_=zq_ps)

            nc.sync.dma_start(out=zq_v[b], in_=zq_b)
            nc.scalar.dma_start(out=idx_v[b], in_=idxt_row)

    nc.compile()
    return nc


_NC = None


def _get_nc():
    global _NC
    if _NC is None:
        _NC = build_nc()
    return _NC


def kernel(**inputs):
    from concourse.bass_utils import run_bass_kernel_spmd

    z = np.ascontiguousarray(np.asarray(inputs["z"], dtype=np.float32))
    emb = np.ascontiguousarray(np.asarray(inputs["embedding"], dtype=np.float32))
    nc = _get_nc()
    in_maps = [
        {"z": z[i * SHARD : (i + 1) * SHARD], "embedding": emb}
        for i in range(N_CORES)
    ]
    res = run_bass_kernel_spmd(nc, in_maps, list(range(N_CORES))).results
    z_q = np.concatenate([r["z_q"] for r in res], axis=0)
    idx = np.concatenate([r["idx"] for r in res], axis=0).astype(np.int32)
    return z_q, idx
